# revision 20
# baseline (speedup 1.0000x reference)
"""Trainium2 Bass kernel for a Tacotron-style decoder step (nn_Decoder).

Contract: kernel(**inputs) takes the FULL unsharded inputs (as produced by
setup_inputs()) and returns the full outputs, matching the reference:
    (mels, scores, attn_h, h1, h2, c1, c2, context, stop)

Strategy: data-parallel over batch. B=64 rows are sharded 8-per-core across
8 NeuronCores; weights are replicated. All compute runs on-device in one
Bass/Tile program per core:
  - PreNet (2xMLP with fixed dropout masks, computed host-side from key 42,
    input-independent) -> GRU cell -> location-sensitive attention
    (the conv branch over the all-zeros cumulative attention folds into a
    constant bias) -> softmax over T -> context matvec against encoder_seq
    -> residual 2-layer LSTM stack -> output projections.

Layouts: small activations are kept feature-major ([feat, batch]) so every
GEMM is a natural lhsT.T @ rhs on the PE; encoder_seq_proj is pre-transposed
host-side to [d, t] so the big tanh uses the per-partition bias port of the
scalar engine; encoder_seq streams through the PE as stationary tiles so the
context lands directly in feature-major layout.
"""

import os
import numpy as np
from contextlib import ExitStack

import concourse.bass as bass
import concourse.bacc as bacc
import concourse.tile as tile
import concourse.mybir as mybir
from concourse.bass_utils import run_bass_kernel_spmd

import ml_dtypes

F32 = mybir.dt.float32
BF16 = mybir.dt.bfloat16
I32 = mybir.dt.int32

# ---- model dims (hardcoded from the problem spec) ----
B, T = 64, 1024
N_MELS, DEC, LSTM_D, CTX = 80, 128, 512, 512
PRE = 256
NCORES = 8
BL = B // NCORES  # 8 rows per core

# config: "f32" = everything fp32; "bf16" = big streams in bf16
CFG = os.environ.get("DEC_KERNEL_CFG", "bf16")


def _np_dt(dt):
    return np.float32 if dt == F32 else ml_dtypes.bfloat16


# ============================================================
# Device program
# ============================================================

def build_program(cfg: str):
    dt_big = BF16 if cfg == "bf16" else F32   # espT / enc / lstm weights
    nc = bacc.Bacc("TRN2", target_bir_lowering=False, debug=False)

    def din(name, shape, dt=F32):
        return nc.dram_tensor(name, list(shape), dt, kind="ExternalInput").ap()

    def dout(name, shape, dt=F32):
        return nc.dram_tensor(name, list(shape), dt, kind="ExternalOutput").ap()

    # big streams
    d_espT = din("espT", [BL, 128, 1024], dt_big)
    d_enc = din("enc", [BL, 8, 128, 512], dt_big)
    d_chars = din("chars", [BL, 1024], I32)
    # small activations (feature-major)
    d_pn = din("pn_T", [80, BL])
    d_ah = din("ah_T", [128, BL])
    d_cv = din("cv_T", [128, 4, BL])
    d_h1 = din("h1_T", [128, 4, BL], dt_big)
    d_h2 = din("h2_T", [128, 4, BL], dt_big)
    d_c1 = din("c1_T", [128, 4, BL])
    d_c2 = din("c2_T", [128, 4, BL])
    d_m1 = din("m1_T", [128, 2, BL])
    d_m2 = din("m2_T", [128, 2, BL])
    # weights (lhsT layout, k-packed)
    d_fc1w = din("fc1_wT", [80, 2, 128])
    d_fc2w = din("fc2_wT", [128, 2, 2, 128])        # [p, k, m, 128]
    d_gruih = din("gru_ihT", [128, 6, 3, 128])
    d_gruhh = din("gru_hhT", [128, 3, 128])
    d_ww = din("Ww_T", [128, 128])
    d_riw = din("ri_wT", [128, 5, 4, 128])
    d_l1ih = din("l1_ihT", [128, 4, 16, 128], dt_big)
    d_l1hh = din("l1_hhT", [128, 4, 16, 128], dt_big)
    d_l2ih = din("l2_ihT", [128, 4, 16, 128], dt_big)
    d_l2hh = din("l2_hhT", [128, 4, 16, 128], dt_big)
    d_mpw = din("mp_wT", [128, 4, 80])
    d_spw = din("sp_wT", [128, 8])
    d_vw = din("v_w", [128, 1], dt_big)
    # biases / consts
    d_fc1b = din("fc1_b", [128, 2])
    d_fc2b = din("fc2_b", [128, 2])
    d_brzh = din("gru_brz_half", [128, 2])   # 0.5*(b_ih+b_hh)[r,z]
    d_bihn = din("gru_bihn", [128, 1])
    d_bhhn = din("gru_bhhn", [128, 1])
    d_pqb = din("pq_b", [128, 1])            # W_b + L_w@conv_b
    d_ribb = din("ri_bb", [128, 4, BL])      # ri_b broadcast over batch
    d_l1bb = din("l1_bb", [128, 16, BL])     # (b_ih+b_hh) bcast
    d_l2bb = din("l2_bb", [128, 16, BL])
    d_spb = din("sp_b_half", [1, 1])
    d_id128 = din("id128", [128, 128])

    # outputs
    o_scores = dout("o_scores", [BL, 1024])
    o_mels = dout("o_mels", [80, BL])
    o_attn = dout("o_attn", [128, BL])
    o_h1 = dout("o_h1", [128, 4, BL])
    o_h2 = dout("o_h2", [128, 4, BL])
    o_c1 = dout("o_c1", [128, 4, BL])
    o_c2 = dout("o_c2", [128, 4, BL])
    o_ctx = dout("o_ctx", [128, 4, BL])
    o_stop = dout("o_stop", [1, BL])

    AF = mybir.ActivationFunctionType

    with tile.TileContext(nc) as tc, ExitStack() as ctx:
        cpool = ctx.enter_context(tc.tile_pool(name="consts", bufs=1))
        work = ctx.enter_context(tc.tile_pool(name="work", bufs=1))

        def cload(dram_ap, dt=None):
            dt = dt or dram_ap.dtype
            t = cpool.tile(list(dram_ap.shape), dt, tag=dram_ap.tensor.name,
                           name=dram_ap.tensor.name + "_sb")
            nc.sync.dma_start(t[:], dram_ap)
            return t

        # ---- stage all small constants ----
        pn = cload(d_pn); ah = cload(d_ah); cv = cload(d_cv)
        h1T = cload(d_h1); h2T = cload(d_h2); c1T = cload(d_c1); c2T = cload(d_c2)
        m1 = cload(d_m1); m2 = cload(d_m2)
        fc1w = cload(d_fc1w); fc2w = cload(d_fc2w)
        gruih = cload(d_gruih); gruhh = cload(d_gruhh); ww = cload(d_ww)
        riw = cload(d_riw)
        mpw = cload(d_mpw); spw = cload(d_spw); vw = cload(d_vw)
        fc1b = cload(d_fc1b); fc2b = cload(d_fc2b)
        brzh = cload(d_brzh); bihn = cload(d_bihn); bhhn = cload(d_bhhn)
        pqb = cload(d_pqb); ribb = cload(d_ribb)
        l1bb = cload(d_l1bb); l2bb = cload(d_l2bb)
        spb = cload(d_spb); id128 = cload(d_id128)
        chars = cpool.tile([BL, 1024], I32, tag="chars", name="chars_sb")
        nc.sync.dma_start(chars[:], d_chars)
        cmask = work.tile([BL, 1024], F32, tag="cmask", name="cmask")
        nc.vector.tensor_scalar(cmask[:], chars[:], 0.0, None,
                                mybir.AluOpType.not_equal)

        # =========================================================
        # Phase A: prenet -> GRU -> pq2   (all [128, BL]-sized)
        # =========================================================
        with tc.tile_pool(name="psA", bufs=2, space="PSUM") as psA:
            # prenet fc1: [80]->[256]
            ps = psA.tile([128, 2, BL], F32, tag="ps")
            for m in range(2):
                nc.tensor.matmul(ps[:, m, :], fc1w[:, m, :], pn[:], start=True, stop=True)
            h1p = work.tile([128, 2, BL], F32, tag="h1p")
            for m in range(2):
                nc.scalar.activation(h1p[:, m, :], ps[:, m, :], AF.Relu,
                                     bias=fc1b[:, m:m + 1])
            nc.vector.tensor_mul(h1p[:], h1p[:], m1[:])

            # prenet fc2: [256]->[256]
            ps2 = psA.tile([128, 2, BL], F32, tag="ps")
            for m in range(2):
                for k in range(2):
                    nc.tensor.matmul(ps2[:, m, :], fc2w[:, k, m, :], h1p[:, k, :],
                                     start=(k == 0), stop=(k == 1))
            h2p = work.tile([128, 2, BL], F32, tag="h2p")
            for m in range(2):
                nc.scalar.activation(h2p[:, m, :], ps2[:, m, :], AF.Relu,
                                     bias=fc2b[:, m:m + 1])
            nc.vector.tensor_mul(h2p[:], h2p[:], m2[:])

            # GRU: gi = [cv(4k), prenet(2k)] @ w_ih.T ; gh = ah @ w_hh.T
            gi = psA.tile([128, 3, BL], F32, tag="ps")
            for m in range(3):
                for k in range(6):
                    rhs = cv[:, k, :] if k < 4 else h2p[:, k - 4, :]
                    nc.tensor.matmul(gi[:, m, :], gruih[:, k, m, :], rhs,
                                     start=(k == 0), stop=(k == 5))
            gh = psA.tile([128, 3, BL], F32, tag="ps2")
            for m in range(3):
                nc.tensor.matmul(gh[:, m, :], gruhh[:, m, :], ah[:], start=True, stop=True)

            # r, z = sigmoid(gi[rz] + gh[rz] + brz)  (sigmoid via tanh)
            ghsb = work.tile([128, 3, BL], F32, tag="ghsb")
            nc.vector.tensor_copy(ghsb[:], gh[:])
            srz = work.tile([128, 2, BL], F32, tag="srz")
            nc.vector.tensor_add(srz[:], gi[:, 0:2, :], ghsb[:, 0:2, :])
            trz = work.tile([128, 2, BL], F32, tag="trz")
            for m in range(2):
                nc.scalar.activation(trz[:, m, :], srz[:, m, :], AF.Tanh,
                                     bias=brzh[:, m:m + 1], scale=0.5)
            nc.vector.tensor_scalar(trz[:], trz[:], 0.5, 0.5,
                                    mybir.AluOpType.mult, mybir.AluOpType.add)
            # n = tanh(gi[n] + bihn + r*(gh[n] + bhhn))
            hn = work.tile([128, BL], F32, tag="hn")
            nc.scalar.activation(hn[:], ghsb[:, 2, :], AF.Identity, bias=bhhn[:, 0:1])
            rn = work.tile([128, BL], F32, tag="rn")
            nc.vector.tensor_mul(rn[:], trz[:, 0, :], hn[:])
            sn = work.tile([128, BL], F32, tag="sn")
            nc.vector.tensor_add(sn[:], gi[:, 2, :], rn[:])
            nt = work.tile([128, BL], F32, tag="nt")
            nc.scalar.activation(nt[:], sn[:], AF.Tanh, bias=bihn[:, 0:1])
            # attn_h = n + z*(h_prev - n)
            hmn = work.tile([128, BL], F32, tag="hmn")
            nc.vector.tensor_sub(hmn[:], ah[:], nt[:])
            zt = work.tile([128, BL], F32, tag="zt")
            nc.vector.tensor_mul(zt[:], trz[:, 1, :], hmn[:])
            attn = work.tile([128, BL], F32, tag="attn")
            nc.vector.tensor_add(attn[:], nt[:], zt[:])
            nc.sync.dma_start(o_attn, attn[:])

            # pq2 = attn @ W_w.T + (W_b + L_w@conv_b)
            pqp = psA.tile([128, BL], F32, tag="ps2")
            nc.tensor.matmul(pqp[:], ww[:], attn[:], start=True, stop=True)
            pq2 = work.tile([128, BL], F32, tag="pq2")
            nc.scalar.activation(pq2[:], pqp[:], AF.Identity, bias=pqb[:, 0:1])

        # =========================================================
        # Phase C-pre: LSTM hidden-side gates (independent of attention)
        # =========================================================
        wstream = ctx.enter_context(tc.tile_pool(name="wstream", bufs=4))

        def big_gemm(pg, d_w, rhs_fn, nm):
            """out[m] accumulates sum_k wT[k, m].T @ rhs_k; streams weight k-chunks."""
            gps = pg.tile([128, 16, BL], F32, tag="g", name=nm + "ps")
            for k in range(4):
                wk = wstream.tile([128, 16, 128], dt_big, tag="wk", name=nm + f"w{k}")
                nc.sync.dma_start(wk[:], d_w[:, k])
                for m in range(16):
                    # one accumulation group per psum bank: start only arms the
                    # 2KB zero-region once; k=0 writes then overwrite, k>0 add
                    nc.tensor.matmul(gps[:, m, :], wk[:, m, :], rhs_fn(k),
                                     start=(k == 0 and m == 0),
                                     stop=(k == 3 and m == 15))
            return gps

        ghs = []
        with tc.tile_pool(name="pgh", bufs=2, space="PSUM") as pgh:
            for d_whh, hT, nm in ((d_l1hh, h1T, "gh1"), (d_l2hh, h2T, "gh2")):
                gps = big_gemm(pgh, d_whh, lambda k, hT=hT: hT[:, k, :], nm)
                gsb = work.tile([128, 16, BL], F32, tag=nm, name=nm)
                nc.vector.tensor_copy(gsb[:], gps[:])
                ghs.append(gsb)
        gh1_sb, gh2_sb = ghs

        # =========================================================
        # Phase B: tanh(espT + pq2[b]) -> u -> softmax -> scoresT
        # =========================================================
        # u in transposed layout: uT[t, (k, b)] = sum_d tanh(espT+pq2)[d, t] v[d]
        # (tanh tile is the stationary operand; all 64 columns share 1 psum bank)
        dt_t = BF16 if dt_big == BF16 else F32
        uT_sb = work.tile([128, 8, BL], F32, tag="uT_sb", name="uT_sb")
        with tc.tile_pool(name="esp", bufs=3) as espp, \
             tc.tile_pool(name="tanhp", bufs=3) as tanhp, \
             tc.tile_pool(name="ppu", bufs=1, space="PSUM") as ppu:
            put = ppu.tile([128, 8, BL], F32, tag="put", name="put")
            for b in range(BL):
                et = espp.tile([128, 1024], dt_big, tag="esp", name=f"esp{b}")
                nc.sync.dma_start(et[:], d_espT[b])
                th = tanhp.tile([128, 1024], dt_t, tag="tanh", name=f"tanh{b}")
                nc.scalar.activation(th[:], et[:], AF.Tanh, bias=pq2[:, b:b + 1])
                for k in range(8):
                    nc.tensor.matmul(put[:, k, b:b + 1],
                                     th[:, k * 128:(k + 1) * 128], vw[:],
                                     start=(b == 0 and k == 0),
                                     stop=(b == BL - 1 and k == 7))
            nc.vector.tensor_copy(uT_sb[:], put[:])

        # transpose uT -> u rows [BL, 1024]
        u_sb = work.tile([BL, 1024], F32, tag="u_sb", name="u_sb")
        with tc.tile_pool(name="ptru", bufs=4, space="PSUM") as ptru:
            for k in range(8):
                tpu = ptru.tile([BL, 128], F32, tag="tpu", name=f"tpu{k}")
                nc.tensor.transpose(tpu[:], uT_sb[:, k, :], id128[:])
                nc.vector.tensor_copy(u_sb[:, k * 128:(k + 1) * 128], tpu[:])

        # softmax over t with char mask (mask multiplies logits, as reference)
        um = work.tile([BL, 1024], F32, tag="um", name="um")
        nc.vector.tensor_mul(um[:], u_sb[:], cmask[:])
        esum = work.tile([BL, 1], F32, tag="esum", name="esum")
        e_sb = work.tile([BL, 1024], F32, tag="e_sb", name="e_sb")
        nc.scalar.activation(e_sb[:], um[:], AF.Exp, accum_out=esum[:])
        rcp = work.tile([BL, 1], F32, tag="rcp", name="rcp")
        nc.vector.reciprocal(rcp[:], esum[:])
        sc_sb = work.tile([BL, 1024], F32, tag="sc_sb", name="sc_sb")
        nc.vector.tensor_scalar(sc_sb[:], e_sb[:], rcp[:], None,
                                mybir.AluOpType.mult)
        nc.sync.dma_start(o_scores, sc_sb[:])

        # transpose scores chunks -> scT [128(t), k, b]
        scT = work.tile([128, 8, BL], dt_big, tag="scT")
        with tc.tile_pool(name="ptr", bufs=4, space="PSUM") as ptr:
            for k in range(8):
                tp = ptr.tile([128, BL], F32, tag="tp", name=f"tp{k}")
                nc.tensor.transpose(tp[:], sc_sb[:, k * 128:(k + 1) * 128],
                                    id128[0:8, 0:8])
                nc.vector.tensor_copy(scT[:, k, :], tp[:])

        # =========================================================
        # context: ctxT[d, j, b] = sum_t enc[b, t, d] * scores[b, t]
        # =========================================================
        ctx_sb = work.tile([128, 4, BL], F32, tag="ctx_sb")
        with tc.tile_pool(name="encp", bufs=6) as encp, \
             tc.tile_pool(name="pctx", bufs=1, space="PSUM") as pctx:
            cps = pctx.tile([128, 4, BL], F32, tag="cps")
            for b in range(BL):
                for k in range(8):
                    et = encp.tile([128, 512], dt_big, tag="enc", name=f"enc{b}_{k}")
                    nc.sync.dma_start(et[:], d_enc[b, k])
                    for j in range(4):
                        nc.tensor.matmul(cps[:, j, b:b + 1],
                                         et[:, j * 128:(j + 1) * 128],
                                         scT[:, k, b:b + 1],
                                         start=(b == 0 and k == 0 and j == 0),
                                         stop=(b == BL - 1 and k == 7 and j == 3))
            nc.vector.tensor_copy(ctx_sb[:], cps[:])
        nc.sync.dma_start(o_ctx, ctx_sb[:])

        # =========================================================
        # Phase D: x = ri(ctx, attn) ; LSTM1 ; LSTM2 ; outputs
        # =========================================================
        def lstm_cell(pgi, x_bf, d_wih, gh_sb, bb, cT, o_c, o_h, nm):
            """returns h_sb [128, 4, BL] (f32)"""
            gps = big_gemm(pgi, d_wih, lambda k: x_bf[:, k, :], nm)
            s1 = work.tile([128, 16, BL], F32, tag=nm + "s1")
            nc.vector.tensor_add(s1[:], gps[:], gh_sb[:])
            nc.vector.tensor_add(s1[:], s1[:], bb[:])
            # i,f tiles 0..7 ; g tiles 8..11 ; o tiles 12..15
            sif = work.tile([128, 8, BL], F32, tag=nm + "sif")
            nc.scalar.activation(sif[:], s1[:, 0:8, :], AF.Tanh, scale=0.5)
            so = work.tile([128, 4, BL], F32, tag=nm + "so")
            nc.scalar.activation(so[:], s1[:, 12:16, :], AF.Tanh, scale=0.5)
            nc.vector.tensor_scalar(sif[:], sif[:], 0.5, 0.5,
                                    mybir.AluOpType.mult, mybir.AluOpType.add)
            nc.vector.tensor_scalar(so[:], so[:], 0.5, 0.5,
                                    mybir.AluOpType.mult, mybir.AluOpType.add)
            gt = work.tile([128, 4, BL], F32, tag=nm + "gt")
            nc.scalar.activation(gt[:], s1[:, 8:12, :], AF.Tanh)
            # c2 = f*c + i*g
            fc = work.tile([128, 4, BL], F32, tag=nm + "fc")
            nc.vector.tensor_mul(fc[:], sif[:, 4:8, :], cT[:])
            ig = work.tile([128, 4, BL], F32, tag=nm + "ig")
            nc.vector.tensor_mul(ig[:], sif[:, 0:4, :], gt[:])
            c2 = work.tile([128, 4, BL], F32, tag=nm + "c2")
            nc.vector.tensor_add(c2[:], fc[:], ig[:])
            nc.sync.dma_start(o_c, c2[:])
            tc2 = work.tile([128, 4, BL], F32, tag=nm + "tc2")
            nc.scalar.activation(tc2[:], c2[:], AF.Tanh)
            h = work.tile([128, 4, BL], F32, tag=nm + "h")
            nc.vector.tensor_mul(h[:], so[:], tc2[:])
            nc.sync.dma_start(o_h, h[:])
            return h

        with tc.tile_pool(name="pgi", bufs=2, space="PSUM") as pgi, \
             tc.tile_pool(name="pout", bufs=2, space="PSUM") as pout:
            # x = cat(ctx, attn) @ ri_w.T + ri_b
            xps = pgi.tile([128, 4, BL], F32, tag="xps")
            for m in range(4):
                for k in range(5):
                    rhs = ctx_sb[:, k, :] if k < 4 else attn[:]
                    nc.tensor.matmul(xps[:, m, :], riw[:, k, m, :], rhs,
                                     start=(k == 0), stop=(k == 4))
            x_sb = work.tile([128, 4, BL], F32, tag="x_sb")
            nc.vector.tensor_add(x_sb[:], xps[:], ribb[:])
            if dt_big == BF16:
                x_mm = work.tile([128, 4, BL], BF16, tag="x_mm")
                nc.vector.tensor_copy(x_mm[:], x_sb[:])
            else:
                x_mm = x_sb

            h1n = lstm_cell(pgi, x_mm, d_l1ih, gh1_sb, l1bb, c1T, o_c1, o_h1, "L1")
            x2 = work.tile([128, 4, BL], F32, tag="x2")
            nc.vector.tensor_add(x2[:], x_sb[:], h1n[:])
            if dt_big == BF16:
                x2_mm = work.tile([128, 4, BL], BF16, tag="x2_mm")
                nc.vector.tensor_copy(x2_mm[:], x2[:])
            else:
                x2_mm = x2

            h2n = lstm_cell(pgi, x2_mm, d_l2ih, gh2_sb, l2bb, c2T, o_c2, o_h2, "L2")
            x3 = work.tile([128, 4, BL], F32, tag="x3")
            nc.vector.tensor_add(x3[:], x2[:], h2n[:])

            # mels = x3 @ mp_w[::20].T
            mps = pout.tile([80, BL], F32, tag="mps")
            for k in range(4):
                nc.tensor.matmul(mps[:], mpw[:, k, :], x3[:, k, :],
                                 start=(k == 0), stop=(k == 3))
            mels = work.tile([80, BL], F32, tag="mels")
            nc.vector.tensor_copy(mels[:], mps[:])
            nc.sync.dma_start(o_mels, mels[:])

            # stop = sigmoid(cat(x3, ctx) @ sp_w.T + sp_b)
            sps = pout.tile([1, BL], F32, tag="sps")
            for k in range(8):
                rhs = x3[:, k, :] if k < 4 else ctx_sb[:, k - 4, :]
                nc.tensor.matmul(sps[:], spw[:, k:k + 1], rhs,
                                 start=(k == 0), stop=(k == 7))
            stp = work.tile([1, BL], F32, tag="stp")
            nc.scalar.activation(stp[:], sps[:], AF.Tanh, bias=spb[:, 0:1], scale=0.5)
            nc.vector.tensor_scalar(stp[:], stp[:], 0.5, 0.5,
                                    mybir.AluOpType.mult, mybir.AluOpType.add)
            nc.sync.dma_start(o_stop, stp[:])

    nc.compile()
    return nc


# ============================================================
# Host-side prep
# ============================================================

def _dropout_masks():
    import jax
    cpu = jax.devices("cpu")[0]
    with jax.default_device(cpu):
        dk1, dk2 = jax.random.split(jax.random.key(42))
        m1 = np.asarray(jax.random.bernoulli(dk1, 0.5, (B, PRE)), np.float32) * 2.0
        m2 = np.asarray(jax.random.bernoulli(dk2, 0.5, (B, PRE)), np.float32) * 2.0
    return m1, m2


def _kpack(wT, nk, nm):
    """[in, out] -> [128, nk, nm, 128]: slice (k, m) is wT[k*128:,(m*128):]."""
    inn, out = wT.shape
    assert inn == nk * 128 and out == nm * 128
    return np.ascontiguousarray(
        wT.reshape(nk, 128, nm, 128).transpose(1, 0, 2, 3))


def _featmajor(x, nk):
    """[BL, nk*128] -> [128, nk, BL]"""
    return np.ascontiguousarray(x.T.reshape(nk, 128, BL).transpose(1, 0, 2))


def _prep_shared(inp, cfg):
    """Weight tensors (identical across cores)."""
    f32 = np.float32
    npdt = _np_dt(BF16 if cfg == "bf16" else F32)
    w = {}
    w["fc1_wT"] = np.ascontiguousarray(
        inp["fc1_w"].T.reshape(80, 2, 128)).astype(f32)
    w["fc2_wT"] = _kpack(inp["fc2_w"].T.astype(f32), 2, 2)
    w["gru_ihT"] = _kpack(inp["gru_w_ih"].T.astype(f32), 6, 3)
    w["gru_hhT"] = np.ascontiguousarray(
        inp["gru_w_hh"].T.reshape(128, 3, 128)).astype(f32)
    w["Ww_T"] = np.ascontiguousarray(inp["W_w"].T).astype(f32)
    w["ri_wT"] = _kpack(inp["ri_w"].T.astype(f32), 5, 4)
    for nm, key in (("l1_ihT", "l1_w_ih"), ("l1_hhT", "l1_w_hh"),
                    ("l2_ihT", "l2_w_ih"), ("l2_hhT", "l2_w_hh")):
        w[nm] = _kpack(inp[key].T.astype(f32), 4, 16).astype(npdt)
    mp_sel = inp["mp_w"][::20].astype(f32)          # [80, 512]
    w["mp_wT"] = np.ascontiguousarray(
        mp_sel.T.reshape(4, 128, 80).transpose(1, 0, 2))
    w["sp_wT"] = np.ascontiguousarray(
        inp["sp_w"][0].astype(f32).reshape(8, 128).T)
    w["v_w"] = np.ascontiguousarray(inp["v_w"][0][:, None]).astype(f32).astype(npdt)
    w["fc1_b"] = np.ascontiguousarray(inp["fc1_b"].reshape(2, 128).T).astype(f32)
    w["fc2_b"] = np.ascontiguousarray(inp["fc2_b"].reshape(2, 128).T).astype(f32)
    bih, bhh = inp["gru_b_ih"].astype(f32), inp["gru_b_hh"].astype(f32)
    w["gru_brz_half"] = np.ascontiguousarray(
        (0.5 * (bih + bhh))[:256].reshape(2, 128).T)
    w["gru_bihn"] = bih[256:][:, None].copy()
    w["gru_bhhn"] = bhh[256:][:, None].copy()
    w["pq_b"] = (inp["W_b"].astype(f32)
                 + inp["L_w"].astype(f32) @ inp["conv_b"].astype(f32))[:, None].copy()
    rib = np.ascontiguousarray(inp["ri_b"].astype(f32).reshape(4, 128).T)
    w["ri_bb"] = np.repeat(rib[:, :, None], BL, axis=2).copy()
    for nm, a, b2 in (("l1_bb", "l1_b_ih", "l1_b_hh"), ("l2_bb", "l2_b_ih", "l2_b_hh")):
        s = (inp[a].astype(f32) + inp[b2].astype(f32)).reshape(16, 128).T
        w[nm] = np.repeat(np.ascontiguousarray(s)[:, :, None], BL, axis=2).copy()
    w["sp_b_half"] = (0.5 * inp["sp_b"].astype(f32)).reshape(1, 1).copy()
    w["id128"] = np.eye(128, dtype=f32)
    return w


def _prep_core(inp, m1k, m2k, core, cfg):
    npdt = _np_dt(BF16 if cfg == "bf16" else F32)
    s = slice(core * BL, (core + 1) * BL)
    f32 = np.float32
    d = {}
    d["espT"] = np.ascontiguousarray(
        np.asarray(inp["encoder_seq_proj"][s]).transpose(0, 2, 1)).astype(npdt)
    d["enc"] = np.ascontiguousarray(
        np.asarray(inp["encoder_seq"][s]).reshape(BL, 8, 128, 512)).astype(npdt)
    d["chars"] = np.asarray(inp["chars"][s]).astype(np.int32)
    d["pn_T"] = np.ascontiguousarray(np.asarray(inp["prenet_in"][s]).T).astype(f32)
    d["ah_T"] = np.ascontiguousarray(np.asarray(inp["attn_hidden"][s]).T).astype(f32)
    d["cv_T"] = _featmajor(np.asarray(inp["context_vec"][s]).astype(f32), 4)
    d["h1_T"] = _featmajor(np.asarray(inp["rnn1_hidden"][s]).astype(f32), 4).astype(npdt)
    d["h2_T"] = _featmajor(np.asarray(inp["rnn2_hidden"][s]).astype(f32), 4).astype(npdt)
    d["c1_T"] = _featmajor(np.asarray(inp["rnn1_cell"][s]).astype(f32), 4)
    d["c2_T"] = _featmajor(np.asarray(inp["rnn2_cell"][s]).astype(f32), 4)
    d["m1_T"] = _featmajor(m1k[s], 2)
    d["m2_T"] = _featmajor(m2k[s], 2)
    return d


_CACHE = {}


def _get_program(cfg):
    if cfg not in _CACHE:
        _CACHE[cfg] = build_program(cfg)
    return _CACHE[cfg]


def kernel(**inputs):
    cfg = CFG
    nc = _get_program(cfg)
    m1k, m2k = _dropout_masks()
    shared = _prep_shared(inputs, cfg)
    in_maps = []
    for core in range(NCORES):
        m = dict(shared)
        m.update(_prep_core(inputs, m1k, m2k, core, cfg))
        in_maps.append(m)

    res = run_bass_kernel_spmd(nc, in_maps, list(range(NCORES)))
    r = res.results

    def gather(name):
        return np.stack([r[c][name] for c in range(NCORES)], axis=0)

    mels = gather("o_mels").transpose(0, 2, 1).reshape(B, N_MELS)[:, :, None]
    scores = gather("o_scores").reshape(B, 1024)[:, None, :]
    attn_h = gather("o_attn").transpose(0, 2, 1).reshape(B, DEC)

    def fm_back(name):   # [cores, 128, 4, BL] -> [B, 512]
        return gather(name).transpose(0, 3, 2, 1).reshape(B, LSTM_D)

    h1 = fm_back("o_h1"); h2 = fm_back("o_h2")
    c1 = fm_back("o_c1"); c2 = fm_back("o_c2")
    context = fm_back("o_ctx")
    stop = gather("o_stop").reshape(B, 1)
    return (mels.astype(np.float32), scores.astype(np.float32),
            attn_h.astype(np.float32), h1.astype(np.float32),
            h2.astype(np.float32), c1.astype(np.float32), c2.astype(np.float32),
            context.astype(np.float32), stop.astype(np.float32))


# revision 24
# speedup vs baseline: 1.9957x; 1.9957x over previous
"""Trainium2 Bass kernel for a Tacotron-style decoder step (nn_Decoder).

Contract: kernel(**inputs) takes the FULL unsharded inputs (as produced by
setup_inputs()) and returns the full outputs, matching the reference:
    (mels, scores, attn_h, h1, h2, c1, c2, context, stop)

Strategy: data-parallel over batch. B=64 rows are sharded 8-per-core across
8 NeuronCores; weights are replicated. All compute runs on-device in one
Bass/Tile program per core:
  - PreNet (2xMLP with fixed dropout masks, computed host-side from key 42,
    input-independent) -> GRU cell -> location-sensitive attention
    (the conv branch over the all-zeros cumulative attention folds into a
    constant bias) -> softmax over T -> context matvec against encoder_seq
    -> residual 2-layer LSTM stack -> output projections.

Layouts: small activations are kept feature-major ([feat, batch]) so every
GEMM is a natural lhsT.T @ rhs on the PE; encoder_seq_proj is pre-transposed
host-side to [d, t] so the big tanh uses the per-partition bias port of the
scalar engine; encoder_seq streams through the PE as stationary tiles so the
context lands directly in feature-major layout.
"""

import os
import numpy as np
from contextlib import ExitStack

import concourse.bass as bass
import concourse.bacc as bacc
import concourse.tile as tile
import concourse.mybir as mybir
from concourse.bass_utils import run_bass_kernel_spmd

import ml_dtypes

F32 = mybir.dt.float32
BF16 = mybir.dt.bfloat16
F16 = mybir.dt.float16
I32 = mybir.dt.int32

# ---- model dims (hardcoded from the problem spec) ----
B, T = 64, 1024
N_MELS, DEC, LSTM_D, CTX = 80, 128, 512, 512
PRE = 256
NCORES = 8
BL = B // NCORES  # 8 rows per core

# config: "f32" = everything fp32; "f16"/"bf16" = big streams in 16-bit.
# fp16 is the default: all tensors here are bounded well inside fp16 range and
# its 11-bit mantissa keeps the worst output error ~1e-3 (bf16 gives ~9e-3).
CFG = os.environ.get("DEC_KERNEL_CFG", "f16")

_BIG_DT = {"f32": F32, "bf16": BF16, "f16": F16}
_BIG_NP = {"f32": np.float32, "bf16": ml_dtypes.bfloat16, "f16": np.float16}


def _np_dt(dt):
    return {F32: np.float32, BF16: ml_dtypes.bfloat16, F16: np.float16}[dt]


# ============================================================
# Device program
# ============================================================

def build_program(cfg: str):
    dt_big = _BIG_DT[cfg]   # espT / enc / lstm weights
    nc = bacc.Bacc("TRN2", target_bir_lowering=False, debug=False)

    def din(name, shape, dt=F32):
        return nc.dram_tensor(name, list(shape), dt, kind="ExternalInput").ap()

    def dout(name, shape, dt=F32):
        return nc.dram_tensor(name, list(shape), dt, kind="ExternalOutput").ap()

    # big streams
    d_espT = din("espT", [BL, 128, 1024], dt_big)
    d_enc = din("enc", [BL, 8, 128, 512], dt_big)
    d_chars = din("chars", [BL, 1024], I32)
    # small activations (feature-major)
    d_pn = din("pn_T", [80, BL])
    d_ah = din("ah_T", [128, BL])
    d_cv = din("cv_T", [128, 4, BL])
    d_h1 = din("h1_T", [128, 4, BL], dt_big)
    d_h2 = din("h2_T", [128, 4, BL], dt_big)
    d_c1 = din("c1_T", [128, 4, BL])
    d_c2 = din("c2_T", [128, 4, BL])
    d_m1 = din("m1_T", [128, 2, BL])
    d_m2 = din("m2_T", [128, 2, BL])
    # weights (lhsT layout, k-packed)
    d_fc1w = din("fc1_wT", [80, 2, 128])
    d_fc2w = din("fc2_wT", [128, 2, 2, 128])        # [p, k, m, 128]
    d_gruih = din("gru_ihT", [128, 6, 3, 128])
    d_gruhh = din("gru_hhT", [128, 3, 128])
    d_ww = din("Ww_T", [128, 128])
    d_riw = din("ri_wT", [128, 5, 4, 128])
    d_l1ih = din("l1_ihT", [128, 4, 16, 128], dt_big)
    d_l1hh = din("l1_hhT", [128, 4, 16, 128], dt_big)
    d_l2ih = din("l2_ihT", [128, 4, 16, 128], dt_big)
    d_l2hh = din("l2_hhT", [128, 4, 16, 128], dt_big)
    d_mpw = din("mp_wT", [128, 4, 80])
    d_spw = din("sp_wT", [128, 8])
    d_vw = din("v_w", [128, 1], dt_big)
    # biases / consts
    d_fc1b = din("fc1_b", [128, 2])
    d_fc2b = din("fc2_b", [128, 2])
    d_brzh = din("gru_brz_half", [128, 2])   # 0.5*(b_ih+b_hh)[r,z]
    d_bihn = din("gru_bihn", [128, 1])
    d_bhhn = din("gru_bhhn", [128, 1])
    d_pqb = din("pq_b", [128, 1])            # W_b + L_w@conv_b
    d_ribb = din("ri_bb", [128, 4, BL])      # ri_b broadcast over batch
    d_l1bb = din("l1_bb", [128, 16, BL])     # (b_ih+b_hh) bcast
    d_l2bb = din("l2_bb", [128, 16, BL])
    d_spb = din("sp_b_half", [1, 1])
    d_id128 = din("id128", [128, 128])

    # outputs
    o_scores = dout("o_scores", [BL, 1024])
    o_mels = dout("o_mels", [80, BL])
    o_attn = dout("o_attn", [128, BL])
    o_h1 = dout("o_h1", [128, 4, BL])
    o_h2 = dout("o_h2", [128, 4, BL])
    o_c1 = dout("o_c1", [128, 4, BL])
    o_c2 = dout("o_c2", [128, 4, BL])
    o_ctx = dout("o_ctx", [128, 4, BL])
    o_stop = dout("o_stop", [1, BL])

    AF = mybir.ActivationFunctionType

    with tile.TileContext(nc) as tc, ExitStack() as ctx:
        cpool = ctx.enter_context(tc.tile_pool(name="consts", bufs=1))
        work = ctx.enter_context(tc.tile_pool(name="work", bufs=1))

        def cload(dram_ap, dt=None):
            dt = dt or dram_ap.dtype
            t = cpool.tile(list(dram_ap.shape), dt, tag=dram_ap.tensor.name,
                           name=dram_ap.tensor.name + "_sb")
            nc.sync.dma_start(t[:], dram_ap)
            return t

        # ---- stage all small constants ----
        pn = cload(d_pn); ah = cload(d_ah); cv = cload(d_cv)
        h1T = cload(d_h1); h2T = cload(d_h2); c1T = cload(d_c1); c2T = cload(d_c2)
        m1 = cload(d_m1); m2 = cload(d_m2)
        fc1w = cload(d_fc1w); fc2w = cload(d_fc2w)
        gruih = cload(d_gruih); gruhh = cload(d_gruhh); ww = cload(d_ww)
        riw = cload(d_riw)
        mpw = cload(d_mpw); spw = cload(d_spw); vw = cload(d_vw)
        fc1b = cload(d_fc1b); fc2b = cload(d_fc2b)
        brzh = cload(d_brzh); bihn = cload(d_bihn); bhhn = cload(d_bhhn)
        pqb = cload(d_pqb); ribb = cload(d_ribb)
        l1bb = cload(d_l1bb); l2bb = cload(d_l2bb)
        spb = cload(d_spb); id128 = cload(d_id128)
        chars = cpool.tile([BL, 1024], I32, tag="chars", name="chars_sb")
        nc.sync.dma_start(chars[:], d_chars)
        cmask = work.tile([BL, 1024], F32, tag="cmask", name="cmask")
        nc.vector.tensor_scalar(cmask[:], chars[:], 0.0, None,
                                mybir.AluOpType.not_equal)

        # =========================================================
        # Phase A: prenet -> GRU -> pq2   (all [128, BL]-sized)
        # =========================================================
        with tc.tile_pool(name="psA", bufs=2, space="PSUM") as psA:
            # prenet fc1: [80]->[256]
            ps = psA.tile([128, 2, BL], F32, tag="ps")
            for m in range(2):
                nc.tensor.matmul(ps[:, m, :], fc1w[:, m, :], pn[:], start=True, stop=True)
            h1p = work.tile([128, 2, BL], F32, tag="h1p")
            for m in range(2):
                nc.scalar.activation(h1p[:, m, :], ps[:, m, :], AF.Relu,
                                     bias=fc1b[:, m:m + 1])
            nc.vector.tensor_mul(h1p[:], h1p[:], m1[:])

            # prenet fc2: [256]->[256]
            ps2 = psA.tile([128, 2, BL], F32, tag="ps")
            for m in range(2):
                for k in range(2):
                    nc.tensor.matmul(ps2[:, m, :], fc2w[:, k, m, :], h1p[:, k, :],
                                     start=(k == 0), stop=(k == 1))
            h2p = work.tile([128, 2, BL], F32, tag="h2p")
            for m in range(2):
                nc.scalar.activation(h2p[:, m, :], ps2[:, m, :], AF.Relu,
                                     bias=fc2b[:, m:m + 1])
            nc.vector.tensor_mul(h2p[:], h2p[:], m2[:])

            # GRU: gi = [cv(4k), prenet(2k)] @ w_ih.T ; gh = ah @ w_hh.T
            gi = psA.tile([128, 3, BL], F32, tag="ps")
            for m in range(3):
                for k in range(6):
                    rhs = cv[:, k, :] if k < 4 else h2p[:, k - 4, :]
                    nc.tensor.matmul(gi[:, m, :], gruih[:, k, m, :], rhs,
                                     start=(k == 0), stop=(k == 5))
            gh = psA.tile([128, 3, BL], F32, tag="ps2")
            for m in range(3):
                nc.tensor.matmul(gh[:, m, :], gruhh[:, m, :], ah[:], start=True, stop=True)

            # r, z = sigmoid(gi[rz] + gh[rz] + brz)  (sigmoid via tanh)
            ghsb = work.tile([128, 3, BL], F32, tag="ghsb")
            nc.vector.tensor_copy(ghsb[:], gh[:])
            srz = work.tile([128, 2, BL], F32, tag="srz")
            nc.vector.tensor_add(srz[:], gi[:, 0:2, :], ghsb[:, 0:2, :])
            trz = work.tile([128, 2, BL], F32, tag="trz")
            for m in range(2):
                nc.scalar.activation(trz[:, m, :], srz[:, m, :], AF.Tanh,
                                     bias=brzh[:, m:m + 1], scale=0.5)
            nc.vector.tensor_scalar(trz[:], trz[:], 0.5, 0.5,
                                    mybir.AluOpType.mult, mybir.AluOpType.add)
            # n = tanh(gi[n] + bihn + r*(gh[n] + bhhn))
            hn = work.tile([128, BL], F32, tag="hn")
            nc.scalar.activation(hn[:], ghsb[:, 2, :], AF.Identity, bias=bhhn[:, 0:1])
            rn = work.tile([128, BL], F32, tag="rn")
            nc.vector.tensor_mul(rn[:], trz[:, 0, :], hn[:])
            sn = work.tile([128, BL], F32, tag="sn")
            nc.vector.tensor_add(sn[:], gi[:, 2, :], rn[:])
            nt = work.tile([128, BL], F32, tag="nt")
            nc.scalar.activation(nt[:], sn[:], AF.Tanh, bias=bihn[:, 0:1])
            # attn_h = n + z*(h_prev - n)
            hmn = work.tile([128, BL], F32, tag="hmn")
            nc.vector.tensor_sub(hmn[:], ah[:], nt[:])
            zt = work.tile([128, BL], F32, tag="zt")
            nc.vector.tensor_mul(zt[:], trz[:, 1, :], hmn[:])
            attn = work.tile([128, BL], F32, tag="attn")
            nc.vector.tensor_add(attn[:], nt[:], zt[:])
            nc.sync.dma_start(o_attn, attn[:])

            # pq2 = attn @ W_w.T + (W_b + L_w@conv_b)
            pqp = psA.tile([128, BL], F32, tag="ps2")
            nc.tensor.matmul(pqp[:], ww[:], attn[:], start=True, stop=True)
            pq2 = work.tile([128, BL], F32, tag="pq2")
            nc.scalar.activation(pq2[:], pqp[:], AF.Identity, bias=pqb[:, 0:1])

        # =========================================================
        # Phase C-pre: LSTM hidden-side gates (independent of attention)
        # =========================================================
        wstream = ctx.enter_context(tc.tile_pool(name="wstream", bufs=4))

        def big_gemm(pg, d_w, rhs_fn, nm):
            """out[m] accumulates sum_k wT[k, m].T @ rhs_k; streams weight k-chunks."""
            gps = pg.tile([128, 16, BL], F32, tag="g", name=nm + "ps")
            for k in range(4):
                wk = wstream.tile([128, 16, 128], dt_big, tag="wk", name=nm + f"w{k}")
                nc.sync.dma_start(wk[:], d_w[:, k])
                for m in range(16):
                    # one accumulation group per psum bank: start only arms the
                    # 2KB zero-region once; k=0 writes then overwrite, k>0 add
                    nc.tensor.matmul(gps[:, m, :], wk[:, m, :], rhs_fn(k),
                                     start=(k == 0 and m == 0),
                                     stop=(k == 3 and m == 15))
            return gps

        ghs = []
        with tc.tile_pool(name="pgh", bufs=2, space="PSUM") as pgh:
            for d_whh, hT, nm in ((d_l1hh, h1T, "gh1"), (d_l2hh, h2T, "gh2")):
                gps = big_gemm(pgh, d_whh, lambda k, hT=hT: hT[:, k, :], nm)
                gsb = work.tile([128, 16, BL], F32, tag=nm, name=nm)
                nc.vector.tensor_copy(gsb[:], gps[:])
                ghs.append(gsb)
        gh1_sb, gh2_sb = ghs

        # =========================================================
        # Phase B: tanh(espT + pq2[b]) -> u -> softmax -> scoresT
        # =========================================================
        # u in transposed layout: uT[t, (k, b)] = sum_d tanh(espT+pq2)[d, t] v[d]
        # (tanh tile is the stationary operand; all 64 columns share 1 psum bank)
        dt_t = dt_big
        uT_sb = work.tile([128, 8, BL], F32, tag="uT_sb", name="uT_sb")
        with tc.tile_pool(name="esp", bufs=3) as espp, \
             tc.tile_pool(name="tanhp", bufs=3) as tanhp, \
             tc.tile_pool(name="ppu", bufs=1, space="PSUM") as ppu:
            put = ppu.tile([128, 8, BL], F32, tag="put", name="put")
            for b in range(BL):
                et = espp.tile([128, 1024], dt_big, tag="esp", name=f"esp{b}")
                nc.sync.dma_start(et[:], d_espT[b])
                th = tanhp.tile([128, 1024], dt_t, tag="tanh", name=f"tanh{b}")
                nc.scalar.activation(th[:], et[:], AF.Tanh, bias=pq2[:, b:b + 1])
                for k in range(8):
                    nc.tensor.matmul(put[:, k, b:b + 1],
                                     th[:, k * 128:(k + 1) * 128], vw[:],
                                     start=(b == 0 and k == 0),
                                     stop=(b == BL - 1 and k == 7))
            nc.vector.tensor_copy(uT_sb[:], put[:])

        # transpose uT -> u rows [BL, 1024]
        u_sb = work.tile([BL, 1024], F32, tag="u_sb", name="u_sb")
        with tc.tile_pool(name="ptru", bufs=4, space="PSUM") as ptru:
            for k in range(8):
                tpu = ptru.tile([BL, 128], F32, tag="tpu", name=f"tpu{k}")
                nc.tensor.transpose(tpu[:], uT_sb[:, k, :], id128[:])
                nc.vector.tensor_copy(u_sb[:, k * 128:(k + 1) * 128], tpu[:])

        # softmax over t with char mask (mask multiplies logits, as reference)
        um = work.tile([BL, 1024], F32, tag="um", name="um")
        nc.vector.tensor_mul(um[:], u_sb[:], cmask[:])
        esum = work.tile([BL, 1], F32, tag="esum", name="esum")
        e_sb = work.tile([BL, 1024], F32, tag="e_sb", name="e_sb")
        nc.scalar.activation(e_sb[:], um[:], AF.Exp, accum_out=esum[:])
        rcp = work.tile([BL, 1], F32, tag="rcp", name="rcp")
        nc.vector.reciprocal(rcp[:], esum[:])
        sc_sb = work.tile([BL, 1024], F32, tag="sc_sb", name="sc_sb")
        nc.vector.tensor_scalar(sc_sb[:], e_sb[:], rcp[:], None,
                                mybir.AluOpType.mult)
        nc.sync.dma_start(o_scores, sc_sb[:])

        # transpose scores chunks -> scT [128(t), k, b]
        scT = work.tile([128, 8, BL], dt_big, tag="scT")
        with tc.tile_pool(name="ptr", bufs=4, space="PSUM") as ptr:
            for k in range(8):
                tp = ptr.tile([128, BL], F32, tag="tp", name=f"tp{k}")
                nc.tensor.transpose(tp[:], sc_sb[:, k * 128:(k + 1) * 128],
                                    id128[0:8, 0:8])
                nc.vector.tensor_copy(scT[:, k, :], tp[:])

        # =========================================================
        # context: ctxT[d, j, b] = sum_t enc[b, t, d] * scores[b, t]
        # =========================================================
        ctx_sb = work.tile([128, 4, BL], F32, tag="ctx_sb")
        with tc.tile_pool(name="encp", bufs=6) as encp, \
             tc.tile_pool(name="pctx", bufs=1, space="PSUM") as pctx:
            cps = pctx.tile([128, 4, BL], F32, tag="cps")
            for b in range(BL):
                for k in range(8):
                    et = encp.tile([128, 512], dt_big, tag="enc", name=f"enc{b}_{k}")
                    nc.sync.dma_start(et[:], d_enc[b, k])
                    for j in range(4):
                        nc.tensor.matmul(cps[:, j, b:b + 1],
                                         et[:, j * 128:(j + 1) * 128],
                                         scT[:, k, b:b + 1],
                                         start=(b == 0 and k == 0 and j == 0),
                                         stop=(b == BL - 1 and k == 7 and j == 3))
            nc.vector.tensor_copy(ctx_sb[:], cps[:])
        nc.sync.dma_start(o_ctx, ctx_sb[:])

        # =========================================================
        # Phase D: x = ri(ctx, attn) ; LSTM1 ; LSTM2 ; outputs
        # =========================================================
        def lstm_cell(pgi, x_bf, d_wih, gh_sb, bb, cT, o_c, o_h, nm):
            """returns h_sb [128, 4, BL] (f32)"""
            gps = big_gemm(pgi, d_wih, lambda k: x_bf[:, k, :], nm)
            s1 = work.tile([128, 16, BL], F32, tag=nm + "s1")
            nc.vector.tensor_add(s1[:], gps[:], gh_sb[:])
            nc.vector.tensor_add(s1[:], s1[:], bb[:])
            # i,f tiles 0..7 ; g tiles 8..11 ; o tiles 12..15
            sif = work.tile([128, 8, BL], F32, tag=nm + "sif")
            nc.scalar.activation(sif[:], s1[:, 0:8, :], AF.Tanh, scale=0.5)
            so = work.tile([128, 4, BL], F32, tag=nm + "so")
            nc.scalar.activation(so[:], s1[:, 12:16, :], AF.Tanh, scale=0.5)
            nc.vector.tensor_scalar(sif[:], sif[:], 0.5, 0.5,
                                    mybir.AluOpType.mult, mybir.AluOpType.add)
            nc.vector.tensor_scalar(so[:], so[:], 0.5, 0.5,
                                    mybir.AluOpType.mult, mybir.AluOpType.add)
            gt = work.tile([128, 4, BL], F32, tag=nm + "gt")
            nc.scalar.activation(gt[:], s1[:, 8:12, :], AF.Tanh)
            # c2 = f*c + i*g
            fc = work.tile([128, 4, BL], F32, tag=nm + "fc")
            nc.vector.tensor_mul(fc[:], sif[:, 4:8, :], cT[:])
            ig = work.tile([128, 4, BL], F32, tag=nm + "ig")
            nc.vector.tensor_mul(ig[:], sif[:, 0:4, :], gt[:])
            c2 = work.tile([128, 4, BL], F32, tag=nm + "c2")
            nc.vector.tensor_add(c2[:], fc[:], ig[:])
            nc.sync.dma_start(o_c, c2[:])
            tc2 = work.tile([128, 4, BL], F32, tag=nm + "tc2")
            nc.scalar.activation(tc2[:], c2[:], AF.Tanh)
            h = work.tile([128, 4, BL], F32, tag=nm + "h")
            nc.vector.tensor_mul(h[:], so[:], tc2[:])
            nc.sync.dma_start(o_h, h[:])
            return h

        with tc.tile_pool(name="pgi", bufs=2, space="PSUM") as pgi, \
             tc.tile_pool(name="pout", bufs=2, space="PSUM") as pout:
            # x = cat(ctx, attn) @ ri_w.T + ri_b
            xps = pgi.tile([128, 4, BL], F32, tag="xps")
            for m in range(4):
                for k in range(5):
                    rhs = ctx_sb[:, k, :] if k < 4 else attn[:]
                    nc.tensor.matmul(xps[:, m, :], riw[:, k, m, :], rhs,
                                     start=(k == 0), stop=(k == 4))
            x_sb = work.tile([128, 4, BL], F32, tag="x_sb")
            nc.vector.tensor_add(x_sb[:], xps[:], ribb[:])
            if dt_big != F32:
                x_mm = work.tile([128, 4, BL], dt_big, tag="x_mm")
                nc.vector.tensor_copy(x_mm[:], x_sb[:])
            else:
                x_mm = x_sb

            h1n = lstm_cell(pgi, x_mm, d_l1ih, gh1_sb, l1bb, c1T, o_c1, o_h1, "L1")
            x2 = work.tile([128, 4, BL], F32, tag="x2")
            nc.vector.tensor_add(x2[:], x_sb[:], h1n[:])
            if dt_big != F32:
                x2_mm = work.tile([128, 4, BL], dt_big, tag="x2_mm")
                nc.vector.tensor_copy(x2_mm[:], x2[:])
            else:
                x2_mm = x2

            h2n = lstm_cell(pgi, x2_mm, d_l2ih, gh2_sb, l2bb, c2T, o_c2, o_h2, "L2")
            x3 = work.tile([128, 4, BL], F32, tag="x3")
            nc.vector.tensor_add(x3[:], x2[:], h2n[:])

            # mels = x3 @ mp_w[::20].T
            mps = pout.tile([80, BL], F32, tag="mps")
            for k in range(4):
                nc.tensor.matmul(mps[:], mpw[:, k, :], x3[:, k, :],
                                 start=(k == 0), stop=(k == 3))
            mels = work.tile([80, BL], F32, tag="mels")
            nc.vector.tensor_copy(mels[:], mps[:])
            nc.sync.dma_start(o_mels, mels[:])

            # stop = sigmoid(cat(x3, ctx) @ sp_w.T + sp_b)
            sps = pout.tile([1, BL], F32, tag="sps")
            for k in range(8):
                rhs = x3[:, k, :] if k < 4 else ctx_sb[:, k - 4, :]
                nc.tensor.matmul(sps[:], spw[:, k:k + 1], rhs,
                                 start=(k == 0), stop=(k == 7))
            stp = work.tile([1, BL], F32, tag="stp")
            nc.scalar.activation(stp[:], sps[:], AF.Tanh, bias=spb[:, 0:1], scale=0.5)
            nc.vector.tensor_scalar(stp[:], stp[:], 0.5, 0.5,
                                    mybir.AluOpType.mult, mybir.AluOpType.add)
            nc.sync.dma_start(o_stop, stp[:])

    nc.compile()
    return nc


# ============================================================
# Host-side prep
# ============================================================

def _dropout_masks():
    import jax
    cpu = jax.devices("cpu")[0]
    with jax.default_device(cpu):
        dk1, dk2 = jax.random.split(jax.random.key(42))
        m1 = np.asarray(jax.random.bernoulli(dk1, 0.5, (B, PRE)), np.float32) * 2.0
        m2 = np.asarray(jax.random.bernoulli(dk2, 0.5, (B, PRE)), np.float32) * 2.0
    return m1, m2


def _kpack(wT, nk, nm):
    """[in, out] -> [128, nk, nm, 128]: slice (k, m) is wT[k*128:,(m*128):]."""
    inn, out = wT.shape
    assert inn == nk * 128 and out == nm * 128
    return np.ascontiguousarray(
        wT.reshape(nk, 128, nm, 128).transpose(1, 0, 2, 3))


def _featmajor(x, nk):
    """[BL, nk*128] -> [128, nk, BL]"""
    return np.ascontiguousarray(x.T.reshape(nk, 128, BL).transpose(1, 0, 2))


def _prep_shared(inp, cfg):
    """Weight tensors (identical across cores)."""
    f32 = np.float32
    npdt = _BIG_NP[cfg]
    w = {}
    w["fc1_wT"] = np.ascontiguousarray(
        inp["fc1_w"].T.reshape(80, 2, 128)).astype(f32)
    w["fc2_wT"] = _kpack(inp["fc2_w"].T.astype(f32), 2, 2)
    w["gru_ihT"] = _kpack(inp["gru_w_ih"].T.astype(f32), 6, 3)
    w["gru_hhT"] = np.ascontiguousarray(
        inp["gru_w_hh"].T.reshape(128, 3, 128)).astype(f32)
    w["Ww_T"] = np.ascontiguousarray(inp["W_w"].T).astype(f32)
    w["ri_wT"] = _kpack(inp["ri_w"].T.astype(f32), 5, 4)
    for nm, key in (("l1_ihT", "l1_w_ih"), ("l1_hhT", "l1_w_hh"),
                    ("l2_ihT", "l2_w_ih"), ("l2_hhT", "l2_w_hh")):
        w[nm] = _kpack(inp[key].T.astype(f32), 4, 16).astype(npdt)
    mp_sel = inp["mp_w"][::20].astype(f32)          # [80, 512]
    w["mp_wT"] = np.ascontiguousarray(
        mp_sel.T.reshape(4, 128, 80).transpose(1, 0, 2))
    w["sp_wT"] = np.ascontiguousarray(
        inp["sp_w"][0].astype(f32).reshape(8, 128).T)
    w["v_w"] = np.ascontiguousarray(inp["v_w"][0][:, None]).astype(f32).astype(npdt)
    w["fc1_b"] = np.ascontiguousarray(inp["fc1_b"].reshape(2, 128).T).astype(f32)
    w["fc2_b"] = np.ascontiguousarray(inp["fc2_b"].reshape(2, 128).T).astype(f32)
    bih, bhh = inp["gru_b_ih"].astype(f32), inp["gru_b_hh"].astype(f32)
    w["gru_brz_half"] = np.ascontiguousarray(
        (0.5 * (bih + bhh))[:256].reshape(2, 128).T)
    w["gru_bihn"] = bih[256:][:, None].copy()
    w["gru_bhhn"] = bhh[256:][:, None].copy()
    w["pq_b"] = (inp["W_b"].astype(f32)
                 + inp["L_w"].astype(f32) @ inp["conv_b"].astype(f32))[:, None].copy()
    rib = np.ascontiguousarray(inp["ri_b"].astype(f32).reshape(4, 128).T)
    w["ri_bb"] = np.repeat(rib[:, :, None], BL, axis=2).copy()
    for nm, a, b2 in (("l1_bb", "l1_b_ih", "l1_b_hh"), ("l2_bb", "l2_b_ih", "l2_b_hh")):
        s = (inp[a].astype(f32) + inp[b2].astype(f32)).reshape(16, 128).T
        w[nm] = np.repeat(np.ascontiguousarray(s)[:, :, None], BL, axis=2).copy()
    w["sp_b_half"] = (0.5 * inp["sp_b"].astype(f32)).reshape(1, 1).copy()
    w["id128"] = np.eye(128, dtype=f32)
    return w


def _prep_core(inp, m1k, m2k, core, cfg):
    npdt = _BIG_NP[cfg]
    s = slice(core * BL, (core + 1) * BL)
    f32 = np.float32
    d = {}
    d["espT"] = np.ascontiguousarray(
        np.asarray(inp["encoder_seq_proj"][s]).transpose(0, 2, 1)).astype(npdt)
    d["enc"] = np.ascontiguousarray(
        np.asarray(inp["encoder_seq"][s]).reshape(BL, 8, 128, 512)).astype(npdt)
    d["chars"] = np.asarray(inp["chars"][s]).astype(np.int32)
    d["pn_T"] = np.ascontiguousarray(np.asarray(inp["prenet_in"][s]).T).astype(f32)
    d["ah_T"] = np.ascontiguousarray(np.asarray(inp["attn_hidden"][s]).T).astype(f32)
    d["cv_T"] = _featmajor(np.asarray(inp["context_vec"][s]).astype(f32), 4)
    d["h1_T"] = _featmajor(np.asarray(inp["rnn1_hidden"][s]).astype(f32), 4).astype(npdt)
    d["h2_T"] = _featmajor(np.asarray(inp["rnn2_hidden"][s]).astype(f32), 4).astype(npdt)
    d["c1_T"] = _featmajor(np.asarray(inp["rnn1_cell"][s]).astype(f32), 4)
    d["c2_T"] = _featmajor(np.asarray(inp["rnn2_cell"][s]).astype(f32), 4)
    d["m1_T"] = _featmajor(m1k[s], 2)
    d["m2_T"] = _featmajor(m2k[s], 2)
    return d


_CACHE = {}


def _get_program(cfg):
    if cfg not in _CACHE:
        _CACHE[cfg] = build_program(cfg)
    return _CACHE[cfg]


def kernel(**inputs):
    cfg = CFG
    nc = _get_program(cfg)
    m1k, m2k = _dropout_masks()
    shared = _prep_shared(inputs, cfg)
    in_maps = []
    for core in range(NCORES):
        m = dict(shared)
        m.update(_prep_core(inputs, m1k, m2k, core, cfg))
        in_maps.append(m)

    res = run_bass_kernel_spmd(nc, in_maps, list(range(NCORES)))
    r = res.results

    def gather(name):
        return np.stack([r[c][name] for c in range(NCORES)], axis=0)

    mels = gather("o_mels").transpose(0, 2, 1).reshape(B, N_MELS)[:, :, None]
    scores = gather("o_scores").reshape(B, 1024)[:, None, :]
    attn_h = gather("o_attn").transpose(0, 2, 1).reshape(B, DEC)

    def fm_back(name):   # [cores, 128, 4, BL] -> [B, 512]
        return gather(name).transpose(0, 3, 2, 1).reshape(B, LSTM_D)

    h1 = fm_back("o_h1"); h2 = fm_back("o_h2")
    c1 = fm_back("o_c1"); c2 = fm_back("o_c2")
    context = fm_back("o_ctx")
    stop = gather("o_stop").reshape(B, 1)
    return (mels.astype(np.float32), scores.astype(np.float32),
            attn_h.astype(np.float32), h1.astype(np.float32),
            h2.astype(np.float32), c1.astype(np.float32), c2.astype(np.float32),
            context.astype(np.float32), stop.astype(np.float32))
